# revision 14
# baseline (speedup 1.0000x reference)
"""Trainium2 Bass kernel for single-head causal attention (v3: parity k-split,
zero collectives).

Problem: B=4, S=2048, E=1024, H=64 fp32.
  q = x@Wq; k = x@Wk; v = x@Wv   (bq/bk are zero per spec; bv re-added
  exactly on host since softmax rows sum to 1)
  out = softmax(causal(q k^T / sqrt(H))) v

Sharding: 8 cores = 4 batch pairs. Within a pair, core parity P owns the
128-key blocks kb with kb % 2 == P -> 20 causal [128k x 512q] score blocks
per core, perfectly balanced. Every core runs the IDENTICAL program; all
asymmetry lives in per-core input data:
  - xt is x^T with adjacent 128-column blocks swapped for odd cores, so
    the fixed t==0 slice of every 512-wide s-tile reads that core's own
    key blocks (queries are then block-permuted; masks are permuted to
    match on the host and outputs un-permuted on the host).
  - diagonal ramp-mask contents encode the per-parity offsets.
Each core emits its partial pv (with a ones-column softmax-denominator
row) for all four q-tiles; the host adds the pair's partials, transposes,
divides, and re-adds bv. No cross-core communication on-chip.

All PE traffic is bf16 (fp32 PSUM accumulation); measured end-to-end
rel err ~5e-3 vs the fp32 reference.
"""

import sys
from contextlib import ExitStack

import numpy as np

if "/opt/trn_rl_repo" not in sys.path:
    sys.path.insert(0, "/opt/trn_rl_repo")

import ml_dtypes

import concourse.bacc as bacc
import concourse.mybir as mybir
import concourse.tile as tile

B, S, E, H = 4, 2048, 1024, 64
NCORES = 8
F32 = mybir.dt.float32
BF16 = mybir.dt.bfloat16
AF = mybir.ActivationFunctionType
BF = ml_dtypes.bfloat16

NEC = E // 128    # 8 contraction chunks of 128
NST = S // 512    # 4 s/q tiles of 512
NPOS = 8          # owned 128-key blocks per core
NDIAG = 2         # ramp-masked (diagonal) positions per q-tile

# packed constants blob layout (bf16 columns)
OFF_WQ = 0                      # [128, 8, 64]
OFF_WKV = OFF_WQ + NEC * H      # [128, 8, 128]
OFF_ID = OFF_WKV + NEC * 128    # [128, 64] (identity for V transpose)
OFF_MSK = OFF_ID + H            # [128, 2, 512]
CONST_W = OFF_MSK + NDIAG * 512


def build_program():
    nc = bacc.Bacc("TRN2", target_bir_lowering=False, debug=False,
                   num_devices=NCORES)

    # xt is host-packed tile-major as [s-tile, partition, ec, 512] so each
    # DMA reads one contiguous 8 KiB block per partition (max efficiency)
    xt_d = nc.dram_tensor("xt", [NST, 128, NEC * 512], BF16,
                          kind="ExternalInput")
    cst_d = nc.dram_tensor("cst", [128, CONST_W], BF16, kind="ExternalInput")
    y_d = nc.dram_tensor("y65", [NST, H + 1, 512], F32, kind="ExternalOutput")

    with tile.TileContext(nc) as tc, ExitStack() as ctx:
        sing = ctx.enter_context(tc.tile_pool(name="sing", bufs=1))
        xpool = ctx.enter_context(tc.tile_pool(name="xpool", bufs=1))
        ppool = ctx.enter_context(tc.tile_pool(name="ppool", bufs=4))
        vpool = ctx.enter_context(tc.tile_pool(name="vpool", bufs=2))
        # PSUM budget (8 banks): qE/scores(2) + qO(2) + kv/ppv(2) + vtr(2)
        psQE = ctx.enter_context(tc.tile_pool(name="psQE", bufs=2,
                                              space="PSUM"))
        psQO = ctx.enter_context(tc.tile_pool(name="psQO", bufs=2,
                                              space="PSUM"))
        psB = ctx.enter_context(tc.tile_pool(name="psB", bufs=2,
                                             space="PSUM"))
        psT = ctx.enter_context(tc.tile_pool(name="psT", bufs=2,
                                             space="PSUM"))

        dram = ctx.enter_context(tc.tile_pool(name="dram", bufs=1,
                                              space="DRAM"))

        # PE warm-up during the input-DMA window: ~6us of accumulating
        # matmuls on a memset tile trips the HAM activity monitor to
        # K=8/8 (2.4 GHz) before the real matmuls arrive. The chains are
        # consumed (copy + DMA to scratch) so they can't be elided.
        warm = sing.tile([128, 640], BF16)
        nc.vector.memset(warm, 0.125)
        wuE = psQE.tile([128, 512], F32, tag="big")
        wuO = psQO.tile([128, 512], F32, tag="bigO")
        NWU = 7
        for i in range(NWU):
            nc.tensor.matmul(wuE, warm[:, 0:128], warm[:, 128:640],
                             start=(i == 0), stop=(i == NWU - 1))
            nc.tensor.matmul(wuO, warm[:, 0:128], warm[:, 128:640],
                             start=(i == 0), stop=(i == NWU - 1))
        wusb = sing.tile([128, 512], BF16)
        nc.vector.tensor_copy(wusb[:, 0:256], wuE[:, 0:256])
        nc.vector.tensor_copy(wusb[:, 256:512], wuO[:, 0:256])
        scratch = dram.tile([128, 512], BF16, tag="wuscratch")
        nc.sync.dma_start(out=scratch, in_=wusb)

        # one packed DMA for all constants; xt split into 4 pipelined DMAs
        # alternating between the two HWDGE queues (sync / scalar).
        cst = sing.tile([128, CONST_W], BF16)
        nc.scalar.dma_start(out=cst, in_=cst_d[:, :])
        wq = cst[:, OFF_WQ:OFF_WKV].rearrange("p (c m) -> p c m", c=NEC)
        wkv = cst[:, OFF_WKV:OFF_ID].rearrange("p (c m) -> p c m", c=NEC)
        ident = cst[:, OFF_ID:OFF_MSK]
        msk = cst[:, OFF_MSK:CONST_W].rearrange("p (d q) -> p d q", d=NDIAG)

        xts = []
        for st in range(NST):
            t = xpool.tile([128, NEC, 512], BF16, tag=f"xt{st}")
            eng = nc.sync if st % 2 == 0 else nc.scalar
            eng.dma_start(out=t, in_=xt_d[st])
            xts.append(t)

        # rows 0:64 = even-ec partial QT, rows 64:128 = odd-ec partial; the
        # partial-sum add is folded into the scores contraction against
        # duplicated [KT; KT] rows.
        qpart = sing.tile([128, S], BF16)
        ktdup = sing.tile([128, NPOS * 128], BF16)
        vt = sing.tile([64, NPOS * 128], BF16)
        v_all = sing.tile([128, NPOS, H + 1], BF16)
        nc.vector.memset(v_all[:, :, H:H + 1], 1.0)

        for st in range(NST):
            xt = xts[st]
            sl = slice(st * 512, (st + 1) * 512)
            # ---- Q projection (col-packed: even ec -> array cols 0:63,
            # odd ec -> cols 64:127; separate PSUM banks so the first-
            # matmul bank clear of one half can't wipe the other) ----
            pqE = psQE.tile([128, 512], F32, tag="big")
            pqO = psQO.tile([128, 512], F32, tag="bigO")
            for ec in range(NEC):
                if ec % 2 == 0:
                    nc.tensor.matmul(pqE[0:64, :], wq[:, ec, :], xt[:, ec, :],
                                     start=(ec == 0), stop=(ec == NEC - 2),
                                     tile_position=(0, 0))
                else:
                    nc.tensor.matmul(pqO[64:128, :], wq[:, ec, :],
                                     xt[:, ec, :],
                                     start=(ec == 1), stop=(ec == NEC - 1),
                                     tile_position=(0, 64))
            nc.vector.tensor_copy(qpart[0:64, sl], pqE[0:64, :])
            nc.vector.tensor_copy(qpart[64:128, sl], pqO[64:128, :])

            # ---- K|V fused projection on own key blocks (t==0 of each
            # 256-column pair; host pre-swapped odd cores' blocks) ----
            pkv = psB.tile([128, 256], F32, tag="kv")
            for ec in range(NEC):
                rhs = xt[:, ec, :].rearrange("p (b t c) -> p t b c",
                                             b=2, t=2, c=128)[:, 0, :, :]
                nc.tensor.matmul(pkv, wkv[:, ec, :], rhs,
                                 start=(ec == 0), stop=(ec == NEC - 1))
            slp = slice(st * 256, (st + 1) * 256)
            nc.vector.tensor_copy(ktdup[0:64, slp], pkv[0:64, :])
            nc.scalar.copy(ktdup[64:128, slp], pkv[0:64, :])
            nc.vector.tensor_copy(vt[:, slp], pkv[64:128, :])
            for j in range(2):
                pos = st * 2 + j
                pt = psT.tile([128, H], BF16, tag="vtr")
                nc.tensor.transpose(pt, vt[:, pos * 128:(pos + 1) * 128],
                                    ident[0:H, 0:H])
                nc.vector.tensor_copy(v_all[:, pos, 0:H], pt)

        # ---- phase 2: attention, q-tiles descending ----
        for qt in reversed(range(NST)):
            npos = 2 * qt + 2
            ppv = psB.tile([H + 1, 512], F32, tag="kv")
            for p in range(npos):
                # alternate score banks across both pools (psQO is idle in
                # phase 2) -> 4-deep rotation, fewer PE-queue stalls
                pool = psQE if p % 2 == 0 else psQO
                ps = pool.tile([128, 512], F32,
                               tag="big" if p % 2 == 0 else "bigO")
                nc.tensor.matmul(ps, ktdup[:, p * 128:(p + 1) * 128],
                                 qpart[:, qt * 512:(qt + 1) * 512],
                                 start=True, stop=True)
                pe = ppool.tile([128, 512], BF16, tag="pexp")
                nc.scalar.activation(pe, ps, AF.Exp, scale=0.125)
                j = p - (npos - NDIAG)
                if j >= 0:
                    nc.vector.tensor_mul(pe, pe, msk[:, j, :])
                nc.tensor.matmul(ppv, v_all[:, p, :], pe,
                                 start=(p == 0), stop=(p == npos - 1))
            pv_sb = vpool.tile([H + 1, 512], F32, tag="pv")
            nc.vector.tensor_copy(pv_sb, ppv)
            nc.sync.dma_start(out=y_d[qt], in_=pv_sb)

    nc.compile()
    return nc


_NC_CACHE = None


def _get_nc():
    global _NC_CACHE
    if _NC_CACHE is None:
        _NC_CACHE = build_program()
    return _NC_CACHE


def make_host_inputs(x, Wq, bq, Wk, bk, Wv, bv):
    """Per-core input maps from the full problem inputs."""
    x = np.asarray(x, np.float32)
    wq = np.asarray(Wq, np.float32).reshape(NEC, 128, H).transpose(1, 0, 2)
    wkv = np.hstack([np.asarray(Wk, np.float32), np.asarray(Wv, np.float32)])
    wkv = wkv.reshape(NEC, 128, 128).transpose(1, 0, 2)
    ident = np.eye(128, dtype=np.float32)[:, :H]

    kk = np.arange(128)[:, None]
    qq = np.arange(512)[None, :]

    maps = []
    for c in range(NCORES):
        b, par = c // 2, c % 2
        xb = x[b]  # [S, E]
        if par:
            # swap adjacent 128-row blocks so own (odd) key blocks sit at
            # the fixed t==0 positions; queries become block-permuted,
            # which the masks (below) and host unpermute account for.
            xb = xb.reshape(8, 2, 128, E)[:, ::-1].reshape(S, E)
        xt = xb.T.reshape(NEC, 128, NST, 512).transpose(2, 1, 0, 3)
        xt = np.ascontiguousarray(xt.reshape(NST, 128, NEC * 512)).astype(BF)

        # position p holds kb = 4*(p//2) + 2*(p%2) + par; masked positions
        # are the last two per q-tile with offsets d = 128*par, 256+128*par
        # against the (possibly permuted) local query coordinate.
        qloc = (qq ^ 128) if par else qq
        ds = [128 * par, 256 + 128 * par]
        msk = np.stack([(qloc >= d + kk) for d in ds], axis=1)

        cstf = np.concatenate([
            wq.reshape(128, NEC * H),
            wkv.reshape(128, NEC * 128),
            ident,
            msk.reshape(128, NDIAG * 512),
        ], axis=1)
        assert cstf.shape[1] == CONST_W
        maps.append({"xt": xt, "cst": cstf.astype(BF)})
    return maps


def run_cores(in_maps, trace=False):
    from concourse.bass_utils import run_bass_kernel_spmd
    nc = _get_nc()
    return run_bass_kernel_spmd(nc, in_maps, list(range(NCORES)), trace=trace)


def finish_host(results, bv):
    """Pair-sum partials + transpose + normalize + bias on host."""
    bv = np.asarray(bv, np.float32)
    out = np.empty((B, S, H), np.float32)
    for b in range(B):
        y0 = results[2 * b]["y65"]        # [NST, 65, 512] natural q order
        y1 = results[2 * b + 1]["y65"]    # odd core: q columns XOR 128
        y1 = y1.reshape(NST, H + 1, 2, 2, 128)[:, :, :, ::-1]
        y1 = y1.reshape(NST, H + 1, 512)
        y65 = y0 + y1
        for qt in range(NST):
            num = y65[qt, 0:H, :]
            den = y65[qt, H, :]
            out[b, qt * 512:(qt + 1) * 512, :] = (num / den).T
    return out + bv


def kernel(x, Wq, bq, Wk, bk, Wv, bv):
    in_maps = make_host_inputs(x, Wq, bq, Wk, bk, Wv, bv)
    res = run_cores(in_maps).results
    return finish_host(res, bv)
